# revision 44
# baseline (speedup 1.0000x reference)
"""Trainium2 Bass kernel for nn_AttentionE.

Computes, per sample i:
    s_i   = sum(d_i)                       # d: (N, 6)
    z_ic  = W * s_i * e_ic + b_c           # e: (N, 5), W scalar, b: (5,)
    a_ic  = exp(tanh(z_ic))
    out_ic = e_ic * a_ic / sum_c(a_ic)     # (eps=1e-7 in ref; negligible, denom >= 5/e)

Sharding: data-parallel over the sample axis across 8 NeuronCores.
On-chip layout: each SBUF partition holds a contiguous run of M samples
(rows stay interleaved, [p, m, c]), so DRAM<->SBUF DMAs are fully
contiguous per partition.

Engine split per tile (HW-tuned 2026-08-07, 133us/pass vs 103-112us
pure-DMA floors; see bench_ab.py + TUNING_NOTES.md):
  DVE    : sum-of-6 reduce, z = s bcast-mul e, sum-of-5 reduce,
           reciprocal (fast approx), out = w * r_bcast (critical path)
  ScalarE: tanh x5 (folds scale W + per-component bias b_c), exp;
           also issues the e-input and output DMAs (ACT HWDGE ring,
           carrying ~21MB/pass; the SP ring carries the 12.6MB d input)
  GpSimd : w = a*e only (Pool engine is slow on HW; keep it off the
           critical path)
Buffer decoupling (DECOUPLE="wo"): w and out get their own tiles so the
e- and z-buffers free right after wmul — without this, the e-input DMA
waits block the ACT sequencer and the ring split loses money.
"""

import sys

import numpy as np

_REPO = "/opt/trn_rl_repo"
if _REPO not in sys.path:
    sys.path.insert(0, _REPO)

from contextlib import ExitStack

import concourse.bacc as bacc
import concourse.bass as bass
import concourse.tile as tile
from concourse import mybir

N_CORES = 8
N_FULL = 4194304
P = 128  # SBUF partitions

# Tunables
M = 512  # samples per partition per tile
BUFS = 3

# Engine assignment for the multiply stages: "vector" or "gpsimd"
Z_ENGINE = "vector"
W_ENGINE = "gpsimd"
OUT_ENGINE = "vector"
# Which engine issues the output DMA: "sync" (SP HWDGE ring, shared with
# inputs) or "scalar" (ACT HWDGE ring, separate from inputs).
ODMA_ENGINE = "scalar"
# Which engine issues the d / e input DMAs ("sync" or "scalar").
DDMA_ENGINE = "sync"
EDMA_ENGINE = "scalar"
# Fused bias path: replace the 5 per-component tanh calls (which fold
# per-c bias) with a bias-broadcast add + ONE full-tile tanh(scale=W).
#   False  - baseline 5x tanh
#   "stt"  - scalar_tensor_tensor (z*W + b_b) on DVE + plain tanh
#   "badd" - tensor_tensor add (z + b_b) on BADD_ENGINE + tanh(scale=W)
FUSED_BIAS = False
BADD_ENGINE = "gpsimd"
# Reciprocal: "accurate" (2 DVE insts) or "fast" (1 inst, ~18 bits).
RECIP = "fast"
# Store the z/a intermediate tile in fp16 (halves SBUF traffic for
# tanh/exp/reduce5/wmul streams; tanh saturates so fp16 range is safe,
# ~1e-3 rel err total, gate is 2e-2). Final out tile stays f32.
Z16 = False
# Buffer decoupling (same ops, separate destinations -> earlier
# buffer-free for the WAR waits that stall the input-DMA sequencers):
#   ""   - in-place (w overwrites e-tile, out overwrites z-tile)
#   "w"  - w = a*e into its own wpool tile (e-buf frees after wmul)
#   "wo" - additionally out into its own obuf tile (z-buf frees after wmul)
DECOUPLE = "wo"
WB = 2  # wpool bufs
OB = 2  # obuf bufs
# Split each output DMA into two halves issued on BOTH HWDGE rings
# (SP + ACT) to balance ring load. +1 DMA instruction per tile.
OSPLIT_RING = False
# Per-pool bufs overrides (None -> use `bufs` argument).
DB = None  # dpool
EB = None  # epool
ZB = None  # zpool
SB = None  # small
# Diagnostic: skip all compute; stream d,e in and copy e back out.
# Measures the pure DMA pipeline floor. NOT a correct kernel.
DMAONLY = False
# Split the out-stage multiply: OUT_ENGINE does components [0,K), 
# SPLIT_ENGINE does [K,5) concurrently. 0 = no split.
OUT_SPLIT_K = 0
SPLIT_ENGINE = "gpsimd"
# Split the d/e input DMAs into two halves so compute starts earlier.
DMA_SPLIT = False
# Pairwise-add reductions instead of tensor_reduce (fewer DVE cycles).
PAIRWISE = False
# Input-DMA grouping: one d/e DMA covers DMA_GROUP compute sub-tiles
# (bigger transfers, fewer dispatches; compute still pipelines at m).
DMA_GROUP = 1
IN_BUFS = 2  # bufs for the grouped input pools
OPB = None  # opool (grouped out) bufs override; None -> IN_BUFS
# Ramp-up: split the first tile into RAMP sub-tiles of m/RAMP samples so the
# pipeline reaches steady state sooner (shorter first serial chain).
RAMP = 1
# Taper-down: split the LAST tile into TAIL sub-tiles (shorter drain chain).
TAIL = 1
# Emit input DMAs PREFETCH tiles ahead of their compute (program-order
# bias toward input lookahead; needs bufs > PREFETCH). 0 = off.
PREFETCH = 0
# Tile scheduler: "" = default (CoreSim-model-based), "asap" = ASAP order.
SCHED = ""
# Register bias const tiles inside the TileContext (Tile-tracked deps)
# instead of pre-TC memsets + an extra all-engine barrier.
BIAS_IN_TC = False
# SBUF pool allocator: "stack" (default) or "queue" (ring; reduces false
# WAR overlap-deps between consecutively-released pool buffers).
POOL_MODE = "stack"

# test.py can flip this to get profile/exec-time back
TRACE = False
LAST = {}


def build_bass(W: float, bvals, S: int, m: int = M, bufs: int = BUFS, repeat: int = 1):
    """Build the single-core SPMD program: d[S,6], e[S,5] -> out[S,5].

    repeat>1 wraps the whole tile pass in a hardware For_i loop (timing
    bench only — re-processes the same data each iteration).
    """
    assert S % (P * m) == 0, (S, P, m)
    T = S // (P * m)
    f32 = mybir.dt.float32
    mult = mybir.AluOpType.mult
    add = mybir.AluOpType.add
    X = mybir.AxisListType.X
    ACT = mybir.ActivationFunctionType

    nc = bacc.Bacc("TRN2", debug=False, num_devices=N_CORES)

    # Register the bias values as const APs so activation(bias=<float>) works.
    for i, v in enumerate(dict.fromkeys(float(x) for x in bvals)):
        t_c = nc.alloc_sbuf_tensor(f"const-bias-{i}", [P, 1], f32)
        nc.gpsimd.memset(t_c.ap(), v)
        nc.const_aps.aps[(f32, v)] = t_c.ap()
    nc.all_engine_barrier()

    if FUSED_BIAS:
        # bias row tile [P, 5]: b_c replicated across partitions.
        # badd mode adds BEFORE tanh's scale=W, so store b/W there:
        # tanh(W*(z + b/W)) == tanh(W*z + b).  (|W| ~ N(0,1); guard tiny.)
        if FUSED_BIAS == "badd":
            assert abs(float(W)) > 1e-30, "degenerate W; use stt mode"
            brow = [float(x) / float(W) for x in bvals]
        else:
            brow = [float(x) for x in bvals]
        btile = nc.alloc_sbuf_tensor("btile", [P, 5], f32)
        for c in range(5):
            nc.gpsimd.memset(btile.ap()[:, c : c + 1], brow[c])
        nc.all_engine_barrier()

    d_ap = nc.dram_tensor("d", [S, 6], f32, kind="ExternalInput").ap()
    e_ap = nc.dram_tensor("e", [S, 5], f32, kind="ExternalInput").ap()
    o_ap = nc.dram_tensor("out", [S, 5], f32, kind="ExternalOutput").ap()

    # [T, P, m*c] views; per partition the data is one contiguous DRAM run.
    d_v = d_ap.rearrange("(t p m) c -> t p (m c)", t=T, p=P, m=m)
    e_v = e_ap.rearrange("(t p m) c -> t p (m c)", t=T, p=P, m=m)
    o_v = o_ap.rearrange("(t p m) c -> t p (m c)", t=T, p=P, m=m)

    z_eng = {"vector": nc.vector, "gpsimd": nc.gpsimd}[Z_ENGINE]
    w_eng = {"vector": nc.vector, "gpsimd": nc.gpsimd}[W_ENGINE]
    out_eng = {"vector": nc.vector, "gpsimd": nc.gpsimd}[OUT_ENGINE]
    odma_eng = {"sync": nc.sync, "scalar": nc.scalar}[ODMA_ENGINE]
    _dma_engs = {"sync": nc.sync, "scalar": nc.scalar, "gpsimd": nc.gpsimd}
    ddma_eng = _dma_engs[DDMA_ENGINE]
    edma_eng = _dma_engs[EDMA_ENGINE]

    g = DMA_GROUP
    assert T % g == 0
    if g > 1:
        # grouped views: sample idx = ((tb*P + p)*g + sub)*m + j
        d_vg = d_ap.rearrange("(tb p n) c -> tb p (n c)", tb=T // g, p=P, n=m * g)
        e_vg = e_ap.rearrange("(tb p n) c -> tb p (n c)", tb=T // g, p=P, n=m * g)
        o_vg_flat = o_ap.rearrange(
            "(tb p n) c -> tb p (n c)", tb=T // g, p=P, n=m * g
        )

    import os as _os

    if SCHED:
        _os.environ["TILE_SCHEDULER"] = SCHED
    else:
        _os.environ.pop("TILE_SCHEDULER", None)
    with tile.TileContext(nc, pool_alloc_mode=POOL_MODE) as tc, ExitStack() as ctx:
        if BIAS_IN_TC:
            cpool = ctx.enter_context(tc.tile_pool(name="cpool", bufs=1))
            for i, v in enumerate(dict.fromkeys(float(x) for x in bvals)):
                ct = cpool.tile([P, 1], f32, tag=f"bias{i}")
                nc.gpsimd.memset(ct[:], v)
                nc.const_aps.aps[(f32, v)] = ct[:]

        dpool = ctx.enter_context(
            tc.tile_pool(name="dpool", bufs=DB or (IN_BUFS if g > 1 else bufs))
        )
        epool = ctx.enter_context(
            tc.tile_pool(name="epool", bufs=EB or (IN_BUFS if g > 1 else bufs))
        )
        zpool = ctx.enter_context(tc.tile_pool(name="zpool", bufs=ZB or bufs))
        if Z16 or DECOUPLE == "wo":
            assert g == 1, "obuf split not implemented for grouped DMA"
            obuf = ctx.enter_context(
                tc.tile_pool(name="obuf", bufs=(ZB or bufs) if Z16 else OB)
            )
        if DECOUPLE:
            wpool = ctx.enter_context(tc.tile_pool(name="wpool", bufs=WB))
        if g > 1:
            opool = ctx.enter_context(tc.tile_pool(name="opool", bufs=OPB or IN_BUFS))
        small = ctx.enter_context(tc.tile_pool(name="small", bufs=SB or bufs))

        def emit(dt_, et, o_dst, mm, z_ext=None):
            """Compute one sub-tile of mm samples/partition; store if o_dst."""
            if DMAONLY:
                if o_dst is not None:
                    odma_eng.dma_start(out=o_dst, in_=et)
                return
            ev = et.rearrange("p (m c) -> p m c", c=5)
            s_t = small.tile([P, mm], f32, tag="s")
            dv3 = dt_.rearrange("p (m c) -> p m c", c=6)
            nc.vector.tensor_reduce(out=s_t[:], in_=dv3, axis=X, op=add)

            if z_ext is None:
                z = zpool.tile(
                    [P, 5 * mm], mybir.dt.float16 if Z16 else f32, tag="z"
                )
                zb = z[:]
            else:
                zb = z_ext
            zv = zb.rearrange("p (m c) -> p m c", c=5)
            s_b = s_t[:].unsqueeze(-1).broadcast_to([P, mm, 5])
            z_eng.tensor_tensor(out=zv, in0=s_b, in1=ev, op=mult)

            if FUSED_BIAS == "stt":
                b_b = btile.ap().unsqueeze(1).broadcast_to([P, mm, 5])
                nc.vector.scalar_tensor_tensor(
                    out=zv, in0=zv, scalar=float(W), in1=b_b, op0=mult, op1=add
                )
                nc.scalar.activation(out=zb, in_=zb, func=ACT.Tanh)
            elif FUSED_BIAS == "badd":
                b_b = btile.ap().unsqueeze(1).broadcast_to([P, mm, 5])
                badd_eng = {"vector": nc.vector, "gpsimd": nc.gpsimd}[BADD_ENGINE]
                badd_eng.tensor_tensor(out=zv, in0=zv, in1=b_b, op=add)
                nc.scalar.activation(
                    out=zb, in_=zb, func=ACT.Tanh, bias=0.0, scale=float(W)
                )
            else:
                for c in range(5):
                    nc.scalar.activation(
                        out=zv[:, :, c],
                        in_=zv[:, :, c],
                        func=ACT.Tanh,
                        bias=float(bvals[c]),
                        scale=float(W),
                    )
            nc.scalar.activation(out=zb, in_=zb, func=ACT.Exp)

            dnm = small.tile([P, mm], f32, tag="dnm")
            nc.vector.tensor_reduce(out=dnm[:], in_=zv, axis=X, op=add)
            r = small.tile([P, mm], f32, tag="r")
            if RECIP == "fast":
                nc.vector.reciprocal_approx_fast(out=r[:], in_=dnm[:])
            else:
                scr = small.tile([P, mm], f32, tag="scr")
                nc.vector.reciprocal_approx_accurate(
                    out=r[:], in_=dnm[:], scratch=scr[:]
                )

            if DECOUPLE:
                wt = wpool.tile([P, 5 * mm], f32, tag="w")
                w_eng.tensor_tensor(out=wt[:], in0=zb, in1=et, op=mult)
                w_v = wt[:].rearrange("p (m c) -> p m c", c=5)
            else:
                w_eng.tensor_tensor(out=et, in0=zb, in1=et, op=mult)
                w_v = ev
            r_b = r[:].unsqueeze(-1).broadcast_to([P, mm, 5])
            if Z16 or DECOUPLE == "wo":
                # separate f32 destination tile for the final product
                ot = obuf.tile([P, 5 * mm], f32, tag="o")
                dst_b = ot[:]
                dst_v = dst_b.rearrange("p (m c) -> p m c", c=5)
            else:
                dst_b, dst_v = zb, zv
            if OUT_SPLIT_K:
                k = OUT_SPLIT_K
                spl_eng = {"vector": nc.vector, "gpsimd": nc.gpsimd}[SPLIT_ENGINE]
                out_eng.tensor_tensor(
                    out=dst_v[:, :, :k], in0=w_v[:, :, :k], in1=r_b[:, :, :k], op=mult
                )
                spl_eng.tensor_tensor(
                    out=dst_v[:, :, k:], in0=w_v[:, :, k:], in1=r_b[:, :, k:], op=mult
                )
            else:
                out_eng.tensor_tensor(out=dst_v, in0=w_v, in1=r_b, op=mult)
            if o_dst is not None:
                if OSPLIT_RING:
                    h = 5 * mm // 2
                    nc.sync.dma_start(out=o_dst[:, :h], in_=dst_b[:, :h])
                    nc.scalar.dma_start(out=o_dst[:, h:], in_=dst_b[:, h:])
                else:
                    odma_eng.dma_start(out=o_dst, in_=dst_b)

        rep_ctx = tc.For_i(0, repeat) if repeat > 1 else None
        if rep_ctx is not None:
            ctx.enter_context(rep_ctx)

        if PREFETCH > 0:
            assert g == 1 and RAMP == 1 and TAIL == 1 and PREFETCH < (DB or bufs)

            def load(t):
                dt_tile = dpool.tile([P, 6 * m], f32, tag="dpool")
                ddma_eng.dma_start(out=dt_tile[:], in_=d_v[t])
                et_tile = epool.tile([P, 5 * m], f32, tag="epool")
                edma_eng.dma_start(out=et_tile[:], in_=e_v[t])
                return dt_tile, et_tile

            loaded = [load(t) for t in range(min(PREFETCH, T))]
            for t in range(T):
                if t + PREFETCH < T:
                    loaded.append(load(t + PREFETCH))
                dt_tile, et_tile = loaded[t]
                emit(dt_tile[:], et_tile[:], o_v[t], m)
            T_done = True
        else:
            T_done = False
        for t in range(T if not T_done else 0):
            bt, sub = divmod(t, g)
            if g > 1:
                if sub == 0:
                    dbig = dpool.tile([P, 6 * m * g], f32)
                    ddma_eng.dma_start(out=dbig[:], in_=d_vg[bt])
                    ebig = epool.tile([P, 5 * m * g], f32)
                    edma_eng.dma_start(out=ebig[:], in_=e_vg[bt])
                    obig = None if DMAONLY else opool.tile([P, 5 * m * g], f32)
                if not DMAONLY:
                    emit(
                        dbig[:, sub * 6 * m : (sub + 1) * 6 * m],
                        ebig[:, sub * 5 * m : (sub + 1) * 5 * m],
                        None,
                        m,
                        z_ext=obig[:, sub * 5 * m : (sub + 1) * 5 * m],
                    )
                if sub == g - 1:
                    odma_eng.dma_start(
                        out=o_vg_flat[bt],
                        in_=ebig[:] if DMAONLY else obig[:],
                    )
            elif (t == 0 and RAMP > 1) or (t == T - 1 and TAIL > 1):
                nsub = RAMP if t == 0 else TAIL
                mr = m // nsub
                for k in range(nsub):
                    dk = dpool.tile([P, 6 * mr], f32, tag="dpool")
                    ddma_eng.dma_start(
                        out=dk[:], in_=d_v[t][:, k * 6 * mr : (k + 1) * 6 * mr]
                    )
                    ek = epool.tile([P, 5 * mr], f32, tag="epool")
                    edma_eng.dma_start(
                        out=ek[:], in_=e_v[t][:, k * 5 * mr : (k + 1) * 5 * mr]
                    )
                    emit(
                        dk[:], ek[:], o_v[t][:, k * 5 * mr : (k + 1) * 5 * mr], mr
                    )
            else:
                dt_tile = dpool.tile([P, 6 * m], f32, tag="dpool")
                ddma_eng.dma_start(out=dt_tile[:], in_=d_v[t])
                et_tile = epool.tile([P, 5 * m], f32, tag="epool")
                edma_eng.dma_start(out=et_tile[:], in_=e_v[t])
                emit(dt_tile[:], et_tile[:], o_v[t], m)

    # Legalize: split multi-wait instructions (HW allows 1 wait/inst).
    nc.compile()
    return nc


def kernel(d, e, W, b):
    from concourse.bass_utils import run_bass_kernel_spmd

    d = np.ascontiguousarray(d, dtype=np.float32)
    e = np.ascontiguousarray(e, dtype=np.float32)
    n = d.shape[0]
    assert n % N_CORES == 0
    s = n // N_CORES

    nc = build_bass(float(np.asarray(W).reshape(-1)[0]), np.asarray(b).tolist(), s)

    in_maps = [
        {"d": d[i * s : (i + 1) * s], "e": e[i * s : (i + 1) * s]}
        for i in range(N_CORES)
    ]
    res = run_bass_kernel_spmd(nc, in_maps, list(range(N_CORES)), trace=TRACE)
    LAST["results"] = res
    out = np.concatenate([res.results[i]["out"] for i in range(N_CORES)], axis=0)
    return out.astype(np.float32)

